# revision 1
# baseline (speedup 1.0000x reference)
"""Trainium2 Bass kernel for nn_ByteModel (4-layer diagonal-SSM byte LM).

Model: x = emb_byte[ids] + emb_pos; L x {LayerNorm -> (Wd,WB,WC) projections ->
selective scan over S with diagonal decay exp(delta*A) -> x + y + h@WDp}; head.

Sharding: 8 cores = 4 batches x 2 D-halves, SPMD (one program, per-core data).
Each core keeps the full residual x[512, 2048] for its batch in [d, t] layout
plus its own-half residual xn[256, 2048]. LayerNorm + stats are duplicated in
the pair; projections produce only the core's 256-wide output slice (weights
pre-sliced host-side); the scan runs 256 d x 16 n recurrences on the DVE
tensor_tensor_scan instruction; after each layer the pair AllGathers the two
updated halves and both cores reload the full residual. The head is computed
fully on every core (PE is cheap); the host keeps one copy per batch.

All matmul operands are float32r (full-rate fp32 on the PE). [1, T] row
broadcasts across partitions are K=1 outer-product matmuls into PSUM; biases
are folded into the matmul accumulation as K=1 outer products with ones rows.
"""
import os
import sys
import numpy as np

for _p in ("/opt/trn_rl_repo", os.path.expanduser("~/.axon_site/_ro/trn_rl_repo")):
    if os.path.isdir(_p) and _p not in sys.path:
        sys.path.insert(0, _p)

import concourse.bass as bass
import concourse.bacc as bacc
import concourse.tile as tile
import concourse.mybir as mybir
import concourse.bass_utils as bass_utils

# All ACT funcs used below (Copy, Exp, Ln) live in one loadable table set;
# the default insertion pass alternates between exp-only and ln-only sets,
# paying a ~2.7us table load per switch. Restrict it to the combined set.
_orig_gat = bacc.get_activation_tables
def _gat_combined(arch):
    tabs = _orig_gat(arch)
    key = "natural_log_exp_and_others"
    if key not in tabs:
        return tabs
    want = set(tabs[key])
    out = {}
    for name, funcs in tabs.items():
        if name == key:
            out[name] = funcs
        else:
            # strip functions the combined set covers, so the chooser can
            # only satisfy them there; keep dict order so set ids still
            # line up with act_info.json
            out[name] = {f for f in funcs if f not in want}
    return out
bacc.get_activation_tables = _gat_combined

dt = mybir.dt
F32, F32R, BF16 = dt.float32, dt.float32r, dt.bfloat16
AOT = mybir.AluOpType
AFT = mybir.ActivationFunctionType

B, S, D, N, L, V = 4, 2048, 512, 16, 4, 256
DH = D // 2          # per-core d-slice width
TB = 512             # time block (free dim per instruction / one PSUM bank)
NTB = S // TB
NDC = D // 128       # 4 d-chunks of the full residual
NMC = DH // 128      # 2 d-chunks of the own slice
EPS = 1e-5
N_CORES = 8
AG_GROUPS = [[0, 1], [2, 3], [4, 5], [6, 7]]

_cache = {}


def _build(ascale, n_cores=N_CORES, use_collectives=True):
    """Build + compile the SPMD program. ascale[l][n] = -exp(logA[l,0,n])."""
    nc = bacc.Bacc("TRN2", target_bir_lowering=False, debug=False,
                   num_devices=n_cores)

    def din(name, shape, dtyp):
        return nc.dram_tensor(name, shape, dtyp, kind="ExternalInput").ap()

    ids_f = din("ids_f", [1, S], F32R)
    iota_v = din("iota_v", [V, 1], F32)
    ones_r = din("ones_r", [1, 128], F32R)      # outer-product lhsT (ones row)
    ones_s = din("ones_s", [1, TB], F32R)       # bias-outer rhs (ones row)
    ones_c = din("ones_c", [128, 1], F32R)      # stats lhsT (ones column)
    embT = din("embT", [V, D], F32R)            # emb_byte [v, d], full
    embO = din("embO", [V, DH], F32R)           # emb_byte own d-slice
    posT = din("posT", [D, S], F32)             # emb_pos.T, full
    posO = din("posO", [DH, S], F32)            # emb_pos.T own slice
    wd_in = din("wd_in", [L, D, DH], F32R)
    bd_in = din("bd_in", [L, 1, DH], F32R)
    wbc_in = din("wbc_in", [L, D, 2 * N], F32R)
    bbc_in = din("bbc_in", [L, 1, 2 * N], F32R)
    wdp_in = din("wdp_in", [L, D, DH], F32R)
    bdp_in = din("bdp_in", [L, 1, DH], F32R)
    gb_in = din("gb_in", [L, 2, D], F32R)       # rows [gamma, beta], full
    gbo_in = din("gbo_in", [L, 2, DH], F32R)    # own slice
    gam_in = din("gam_in", [L, D, 1], F32)      # full gamma columns
    gamo_in = din("gamo_in", [L, DH, 1], F32)   # own gamma columns
    sel_in = din("sel_in", [2 * N, 2 * N * 128], F32R)
    whT = din("whT", [D, V], F32R)
    bh_in = din("bh_in", [1, V], F32R)

    logits_out = nc.dram_tensor("logits_full", [S, V], F32,
                                kind="ExternalOutput").ap()

    with tile.TileContext(nc) as tc:
        gp_cm = tc.tile_pool(name="gp", bufs=1)
        gp = gp_cm.__enter__()
        # persistent: full residual (global d-order) + own-half residual
        x_t = [gp.tile([128, S], F32R, tag=f"x{dc}", name=f"x{dc}") for dc in range(NDC)]
        xn_t = [gp.tile([128, S], F32R, tag=f"xn{mc}", name=f"xn{mc}") for mc in range(NMC)]
        ones_r_t = gp.tile([1, 128], F32R, tag="ones_r", name="ones_r")
        ones_s_t = gp.tile([1, TB], F32R, tag="ones_s", name="ones_s")
        ones_c_t = gp.tile([128, 1], F32R, tag="ones_c", name="ones_c")
        sel_t = gp.tile([2 * N, 2 * N * 128], F32R, tag="sel", name="sel")
        nc.sync.dma_start(sel_t[:], sel_in[:])
        eps_t = gp.tile([128, 1], F32, tag="eps", name="eps")
        nc.vector.memset(eps_t[:], EPS)
        nc.sync.dma_start(ones_r_t[:], ones_r[:])
        nc.sync.dma_start(ones_s_t[:], ones_s[:])
        nc.sync.dma_start(ones_c_t[:], ones_c[:])

        dramp_cm = tc.tile_pool(name="dram", bufs=1, space="DRAM")
        dramp = dramp_cm.__enter__()
        ag_in = [dramp.tile([DH, S], F32R, tag=f"agi{l}", name=f"agi{l}") for l in range(L)]
        ag_out = [dramp.tile([D, S], F32R, tag=f"ago{l}", name=f"ago{l}") for l in range(L)]

        # ---------------- embedding: x0 = emb_byte[ids] + emb_pos ----------
        with tc.tile_pool(name="emb_sb", bufs=1) as esb, \
             tc.tile_pool(name="emb_ps", bufs=2, space="PSUM") as eps:
            ids_t = esb.tile([1, S], F32R, tag="ids", name="ids")
            nc.sync.dma_start(ids_t[:], ids_f[:])
            iota_t = [esb.tile([128, 1], F32, tag=f"iota{vc}", name=f"iota{vc}") for vc in range(2)]
            emb_t = [esb.tile([128, D], F32R, tag=f"emb{vc}", name=f"emb{vc}") for vc in range(2)]
            embo_t = [esb.tile([128, DH], F32R, tag=f"embo{vc}", name=f"embo{vc}") for vc in range(2)]
            for vc in range(2):
                vsl = slice(vc * 128, (vc + 1) * 128)
                nc.sync.dma_start(iota_t[vc][:], iota_v[vsl, :])
                nc.sync.dma_start(emb_t[vc][:], embT[vsl, :])
                nc.sync.dma_start(embo_t[vc][:], embO[vsl, :])
            oh_t = [esb.tile([128, S], F32R, tag=f"oh{vc}", name=f"oh{vc}") for vc in range(2)]
            for vc in range(2):
                for tb in range(NTB):
                    sl = slice(tb * TB, (tb + 1) * TB)
                    rep = eps.tile([128, TB], F32, tag="idrep", name="idrep")
                    nc.tensor.matmul(rep[:], ones_r_t[:], ids_t[:, sl],
                                     start=True, stop=True)
                    nc.vector.tensor_scalar(oh_t[vc][:, sl], rep[:],
                                            iota_t[vc][:], None, AOT.is_equal)
            pos_t = [esb.tile([128, S], F32, tag=f"pos{dc}", name=f"pos{dc}") for dc in range(NDC)]
            poso_t = [esb.tile([128, S], F32, tag=f"poso{mc}", name=f"poso{mc}") for mc in range(NMC)]
            for dc in range(NDC):
                nc.sync.dma_start(pos_t[dc][:], posT[dc * 128:(dc + 1) * 128, :])
            for mc in range(NMC):
                nc.sync.dma_start(poso_t[mc][:], posO[mc * 128:(mc + 1) * 128, :])
            for dc in range(NDC):
                for tb in range(NTB):
                    sl = slice(tb * TB, (tb + 1) * TB)
                    x0p = eps.tile([128, TB], F32, tag="x0", name="x0")
                    for vc in range(2):
                        nc.tensor.matmul(
                            x0p[:], emb_t[vc][:, dc * 128:(dc + 1) * 128],
                            oh_t[vc][:, sl], start=(vc == 0), stop=(vc == 1))
                    nc.vector.tensor_add(x_t[dc][:, sl], pos_t[dc][:, sl], x0p[:])
            for mc in range(NMC):
                for tb in range(NTB):
                    sl = slice(tb * TB, (tb + 1) * TB)
                    x0p = eps.tile([128, TB], F32, tag="x0", name="x0")
                    for vc in range(2):
                        nc.tensor.matmul(
                            x0p[:], embo_t[vc][:, mc * 128:(mc + 1) * 128],
                            oh_t[vc][:, sl], start=(vc == 0), stop=(vc == 1))
                    nc.vector.tensor_add(xn_t[mc][:, sl], poso_t[mc][:, sl],
                                         x0p[:])

        # ---------------- layers ------------------------------------------
        for l in range(L):
            with tc.tile_pool(name=f"ly{l}", bufs=1) as lsb:
                hn_t = [lsb.tile([128, S], F32R, tag=f"hn{dc}", name=f"hn{dc}")
                        for dc in range(NDC)]
                dl_t = [lsb.tile([128, S], F32, tag=f"dl{mc}", name=f"dl{mc}")
                        for mc in range(NMC)]
                u_t = [lsb.tile([128, S], F32, tag=f"u{mc}", name=f"u{mc}")
                       for mc in range(NMC)]
                y_t = [lsb.tile([128, S], F32, tag=f"y{mc}", name=f"y{mc}")
                       for mc in range(NMC)]
                bct_t = lsb.tile([2 * N, S], F32R, tag="bct", name="bct")
                wd_t = [lsb.tile([128, DH], F32R, tag=f"wd{kc}", name=f"wd{kc}")
                        for kc in range(NDC)]
                wbc_t = [lsb.tile([128, 2 * N], F32R, tag=f"wbc{kc}", name=f"wbc{kc}")
                         for kc in range(NDC)]
                wdp_t = [lsb.tile([128, DH], F32R, tag=f"wdp{kc}", name=f"wdp{kc}")
                         for kc in range(NDC)]
                for kc in range(NDC):
                    ksl = slice(kc * 128, (kc + 1) * 128)
                    nc.sync.dma_start(wd_t[kc][:], wd_in[l, ksl, :])
                    nc.sync.dma_start(wbc_t[kc][:], wbc_in[l, ksl, :])
                    nc.sync.dma_start(wdp_t[kc][:], wdp_in[l, ksl, :])
                bd_t = lsb.tile([1, DH], F32R, tag="bd", name="bd")
                bbc_t = lsb.tile([1, 2 * N], F32R, tag="bbc", name="bbc")
                bdp_t = lsb.tile([1, DH], F32R, tag="bdp", name="bdp")
                nc.sync.dma_start(bd_t[:], bd_in[l, :, :])
                nc.sync.dma_start(bbc_t[:], bbc_in[l, :, :])
                nc.sync.dma_start(bdp_t[:], bdp_in[l, :, :])
                ga_t = lsb.tile([1, D], F32R, tag="ga", name="ga")
                be_t = lsb.tile([1, D], F32R, tag="be", name="be")
                gao_t = lsb.tile([1, DH], F32R, tag="gao", name="gao")
                beo_t = lsb.tile([1, DH], F32R, tag="beo", name="beo")
                nc.sync.dma_start(ga_t[:], gb_in[l, 0:1, :])
                nc.sync.dma_start(be_t[:], gb_in[l, 1:2, :])
                nc.sync.dma_start(gao_t[:], gbo_in[l, 0:1, :])
                nc.sync.dma_start(beo_t[:], gbo_in[l, 1:2, :])
                gam_t = [lsb.tile([128, 1], F32, tag=f"gam{dc}", name=f"gam{dc}")
                         for dc in range(NDC)]
                gamo_t = [lsb.tile([128, 1], F32, tag=f"gamo{mc}", name=f"gamo{mc}")
                          for mc in range(NMC)]
                for dc in range(NDC):
                    nc.sync.dma_start(gam_t[dc][:],
                                      gam_in[l, dc * 128:(dc + 1) * 128, :])
                for mc in range(NMC):
                    nc.sync.dma_start(gamo_t[mc][:],
                                      gamo_in[l, mc * 128:(mc + 1) * 128, :])

                # ---- LayerNorm (full D for matmul rhs, own half for u) ----
                with tc.tile_pool(name=f"ln{l}", bufs=2) as tsb, \
                     tc.tile_pool(name=f"lnp{l}", bufs=1, space="PSUM") as tp1, \
                     tc.tile_pool(name=f"lnp2{l}", bufs=2, space="PSUM") as tp2:
                    for tb in range(NTB):
                        sl = slice(tb * TB, (tb + 1) * TB)
                        s1p = tp1.tile([1, TB], F32, tag="s1", name="s1")
                        s2p = tp1.tile([1, TB], F32, tag="s2", name="s2")
                        xsq = [None] * NDC
                        for dc in range(NDC):
                            xsq[dc] = tsb.tile([128, TB], F32R, tag="xsq", name="xsq")
                            nc.scalar.activation(xsq[dc][:],
                                                 x_t[dc][:, sl].bitcast(F32),
                                                 AFT.Square)
                        for dc in range(NDC):
                            nc.tensor.matmul(s1p[:], ones_c_t[:], x_t[dc][:, sl],
                                             start=(dc == 0), stop=(dc == NDC - 1))
                        for dc in range(NDC):
                            nc.tensor.matmul(s2p[:], ones_c_t[:], xsq[dc][:],
                                             start=(dc == 0), stop=(dc == NDC - 1))
                        mneg = tsb.tile([1, TB], F32, tag="row", name="mneg",
                                        bufs=6)
                        nc.scalar.activation(mneg[:], s1p[:], AFT.Copy,
                                             scale=-1.0 / D)
                        msq = tsb.tile([1, TB], F32, tag="row", name="msq",
                                       bufs=6)
                        nc.gpsimd.tensor_mul(msq[:], mneg[:], mneg[:])
                        var = tsb.tile([1, TB], F32, tag="row", name="var",
                                       bufs=6)
                        nc.vector.scalar_tensor_tensor(var[:], s2p[:], 1.0 / D,
                                                       msq[:], AOT.mult,
                                                       AOT.subtract)
                        lv = tsb.tile([1, TB], F32, tag="row", name="lv",
                                      bufs=6)
                        nc.scalar.activation(lv[:], var[:], AFT.Ln,
                                             bias=eps_t[:1, :])
                        rstd = tsb.tile([1, TB], F32, tag="row", name="rstd",
                                        bufs=6)
                        nc.scalar.activation(rstd[:], lv[:], AFT.Exp,
                                             scale=-0.5)
                        rstd_r = tsb.tile([1, TB], F32R, tag="row",
                                          name="rstd_r", bufs=6)
                        nc.vector.tensor_copy(rstd_r[:], rstd[:])
                        srep = tp1.tile([128, TB], F32, tag="srep", name="srep")
                        nc.tensor.matmul(srep[:], ones_r_t[:], rstd_r[:],
                                         start=True, stop=True)
                        negms = tsb.tile([1, TB], F32R, tag="row",
                                         name="negms", bufs=6)
                        nc.vector.tensor_mul(negms[:], mneg[:], rstd[:])
                        for dc in range(NDC):
                            gbp = tp2.tile([128, TB], F32, tag="gbp", name="gbp")
                            dsl2 = slice(dc * 128, (dc + 1) * 128)
                            nc.tensor.matmul(gbp[:], ga_t[:, dsl2], negms[:],
                                             start=True, stop=False)
                            nc.tensor.matmul(gbp[:], be_t[:, dsl2], ones_s_t[:],
                                             start=False, stop=True)
                            t1 = tsb.tile([128, TB], F32, tag="t1", name="t1")
                            nc.vector.tensor_mul(
                                t1[:], x_t[dc][:, sl].bitcast(F32), srep[:])
                            nc.vector.scalar_tensor_tensor(
                                hn_t[dc][:, sl], t1[:], gam_t[dc][:], gbp[:],
                                AOT.mult, AOT.add)
                        # own-half normalized copy (for u = delta*h), from xn
                        for mc in range(NMC):
                            gbp = tp2.tile([128, TB], F32, tag="gbp", name="gbp")
                            msl2 = slice(mc * 128, (mc + 1) * 128)
                            nc.tensor.matmul(gbp[:], gao_t[:, msl2], negms[:],
                                             start=True, stop=False)
                            nc.tensor.matmul(gbp[:], beo_t[:, msl2], ones_s_t[:],
                                             start=False, stop=True)
                            t1 = tsb.tile([128, TB], F32, tag="t1", name="t1")
                            nc.vector.tensor_mul(
                                t1[:], xn_t[mc][:, sl].bitcast(F32), srep[:])
                            # h_own lands in u_t; the projection phase turns
                            # it into u = delta * h_own in place
                            nc.vector.scalar_tensor_tensor(
                                u_t[mc][:, sl], t1[:], gamo_t[mc][:], gbp[:],
                                AOT.mult, AOT.add)

                # ---- projections: delta, B, C; u = delta * h_own ----
                with tc.tile_pool(name=f"pj{l}", bufs=3) as psb, \
                     tc.tile_pool(name=f"pjp{l}", bufs=2, space="PSUM") as pps:
                    for tb in range(NTB):
                        sl = slice(tb * TB, (tb + 1) * TB)
                        for mc in range(NMC):
                            msl = slice(mc * 128, (mc + 1) * 128)
                            zp = pps.tile([128, TB], F32, tag="z", name="z")
                            for kc in range(NDC):
                                nc.tensor.matmul(zp[:], wd_t[kc][:, msl],
                                                 hn_t[kc][:, sl],
                                                 start=(kc == 0), stop=False)
                            nc.tensor.matmul(zp[:], bd_t[:, msl], ones_s_t[:],
                                             start=False, stop=True)
                            ez = psb.tile([128, TB], F32, tag="ez", name="ez")
                            nc.scalar.activation(ez[:], zp[:], AFT.Exp)
                            nc.scalar.activation(dl_t[mc][:, sl], ez[:], AFT.Ln,
                                                 bias=1.0)
                            nc.gpsimd.tensor_mul(u_t[mc][:, sl],
                                                 dl_t[mc][:, sl],
                                                 u_t[mc][:, sl])
                        bcp = pps.tile([2 * N, TB], F32, tag="bc", name="bc")
                        for kc in range(NDC):
                            nc.tensor.matmul(bcp[:], wbc_t[kc][:],
                                             hn_t[kc][:, sl],
                                             start=(kc == 0), stop=False)
                        nc.tensor.matmul(bcp[:], bbc_t[:], ones_s_t[:],
                                         start=False, stop=True)
                        nc.vector.tensor_copy(bct_t[:, sl], bcp[:])

                # ---- scan + y; then WDp + residual ----
                with tc.tile_pool(name=f"sc{l}", bufs=2) as ssb, \
                     tc.tile_pool(name=f"scp{l}", bufs=2, space="PSUM") as sps:
                    s_prev = [None] * NMC
                    for n in range(N):
                        for tb in range(NTB):
                            sl = slice(tb * TB, (tb + 1) * TB)
                            brep = sps.tile([128, TB], F32, tag="brep", name="brep",
                                            bufs=3)
                            nc.tensor.matmul(
                                brep[:], sel_t[:, n * 128:(n + 1) * 128],
                                bct_t[:, sl], start=True, stop=True)
                            if n % 2 == 1:
                                brep_sb = ssb.tile([128, TB], F32,
                                                   tag="brep_sb",
                                                   name="brep_sb", bufs=2)
                                nc.scalar.copy(brep_sb[:], brep[:])
                            crep = sps.tile([128, TB], F32, tag="crep", name="crep",
                                            bufs=3)
                            nc.tensor.matmul(
                                crep[:], sel_t[:, (N + n) * 128:(N + n + 1) * 128],
                                bct_t[:, sl], start=True, stop=True)
                            for mc in range(NMC):
                                a_t = ssb.tile([128, TB], F32, tag=f"a{mc}", name=f"a{mc}", bufs=1)
                                nc.scalar.activation(a_t[:], dl_t[mc][:, sl],
                                                     AFT.Exp,
                                                     scale=float(ascale[l][n]))
                                bt_t = ssb.tile([128, TB], F32, tag=f"bt{mc}", name=f"bt{mc}")
                                if n % 2 == 1:
                                    nc.gpsimd.tensor_mul(bt_t[:],
                                                         u_t[mc][:, sl],
                                                         brep_sb[:])
                                else:
                                    nc.vector.tensor_mul(bt_t[:],
                                                         u_t[mc][:, sl],
                                                         brep[:])
                                st_t = ssb.tile([128, TB], F32, tag=f"st{mc}",
                                                bufs=2, name=f"st{mc}")
                                init = 0.0 if tb == 0 else \
                                    s_prev[mc][:, TB - 1:TB]
                                nc.vector.tensor_tensor_scan(
                                    st_t[:], a_t[:], bt_t[:], init,
                                    AOT.mult, AOT.add)
                                s_prev[mc] = st_t
                                if n == 0:
                                    nc.vector.tensor_mul(y_t[mc][:, sl],
                                                         st_t[:], crep[:])
                                else:
                                    cm_t = ssb.tile([128, TB], F32,
                                                    tag=f"cm{mc}", name=f"cm{mc}",
                                                    bufs=2)
                                    nc.vector.tensor_mul(cm_t[:], st_t[:],
                                                         crep[:])
                                    nc.gpsimd.tensor_add(y_t[mc][:, sl],
                                                         y_t[mc][:, sl],
                                                         cm_t[:])

                    for tb in range(NTB):
                        sl = slice(tb * TB, (tb + 1) * TB)
                        for mc in range(NMC):
                            msl = slice(mc * 128, (mc + 1) * 128)
                            dpp = sps.tile([128, TB], F32, tag="dp", name="dp")
                            for kc in range(NDC):
                                nc.tensor.matmul(dpp[:], wdp_t[kc][:, msl],
                                                 hn_t[kc][:, sl],
                                                 start=(kc == 0), stop=False)
                            nc.tensor.matmul(dpp[:], bdp_t[:, msl], ones_s_t[:],
                                             start=False, stop=True)
                            t2 = ssb.tile([128, TB], F32, tag="t2", name="t2",
                                          bufs=1)
                            nc.gpsimd.tensor_add(t2[:],
                                                 xn_t[mc][:, sl].bitcast(F32),
                                                 y_t[mc][:, sl])
                            nc.vector.tensor_add(xn_t[mc][:, sl], t2[:], dpp[:])
                for mc in range(NMC):
                    nc.sync.dma_start(ag_in[l][mc * 128:(mc + 1) * 128, :],
                                      xn_t[mc][:])
                if use_collectives:
                    nc.gpsimd.collective_compute(
                        "AllGather", AOT.bypass, replica_groups=AG_GROUPS,
                        ins=[ag_in[l].opt()], outs=[ag_out[l].opt()])
                else:
                    nc.sync.dma_start(ag_out[l][0:DH, :], ag_in[l][:])
                    nc.sync.dma_start(ag_out[l][DH:D, :], ag_in[l][:])
                for dc in range(NDC):
                    for tb in range(NTB):
                        sl = slice(tb * TB, (tb + 1) * TB)
                        nc.sync.dma_start(x_t[dc][:, sl],
                                          ag_out[l][dc * 128:(dc + 1) * 128, sl])

        # ---------------- head (full S on every core) ----------------------
        with tc.tile_pool(name="hd", bufs=3) as hsb, \
             tc.tile_pool(name="hdp", bufs=2, space="PSUM") as hps:
            wh_t = [hsb.tile([128, V], F32R, tag=f"wh{kc}", bufs=1, name=f"wh{kc}")
                    for kc in range(NDC)]
            for kc in range(NDC):
                nc.sync.dma_start(wh_t[kc][:], whT[kc * 128:(kc + 1) * 128, :])
            bh_t = hsb.tile([1, V], F32R, tag="bh", bufs=1, name="bh")
            nc.sync.dma_start(bh_t[:], bh_in[:])
            for tch in range(S // 128):
                t0 = tch * 128
                hp = hps.tile([128, V], F32, tag="hp", name="hp")
                for kc in range(NDC):
                    nc.tensor.matmul(hp[:], x_t[kc][:, t0:t0 + 128], wh_t[kc][:],
                                     start=(kc == 0), stop=False)
                nc.tensor.matmul(hp[:], ones_r_t[:], bh_t[:],
                                 start=False, stop=True)
                lo = hsb.tile([128, V], F32, tag="lo", name="lo")
                nc.scalar.copy(lo[:], hp[:])
                nc.sync.dma_start(logits_out[t0:t0 + 128, :], lo[:])

        dramp_cm.__exit__(None, None, None)
        gp_cm.__exit__(None, None, None)

    nc.compile()
    return nc


def kernel(byte_ids, emb_byte, emb_pos, logA, Wd, bd, WB, bB, WC, bC,
           WDp, bDp, gamma, beta, Wh, bh):
    byte_ids = np.asarray(byte_ids)
    f32 = lambda a: np.ascontiguousarray(np.asarray(a), dtype=np.float32)
    emb_byte, emb_pos, logA = f32(emb_byte), f32(emb_pos), f32(logA)
    Wd, bd, WB, bB, WC, bC = map(f32, (Wd, bd, WB, bB, WC, bC))
    WDp, bDp, gamma, beta, Wh, bh = map(f32, (WDp, bDp, gamma, beta, Wh, bh))

    ascale = [[-float(np.exp(logA[l, 0, n])) for n in range(N)]
              for l in range(L)]
    key = repr(ascale)
    if key not in _cache:
        _cache[key] = _build(ascale)
    nc = _cache[key]

    wbc = np.concatenate([WB, WC], axis=2)              # [L, D, 2N]
    bbc = np.concatenate([bB, bC], axis=1)[:, None, :]  # [L, 1, 2N]
    gb = np.stack([gamma, beta], axis=1)                # [L, 2, D]
    posT_full = np.ascontiguousarray(emb_pos[:S].T)     # [D, S]
    sel = np.zeros((2 * N, 2 * N * 128), np.float32)
    for n in range(2 * N):
        sel[n, n * 128:(n + 1) * 128] = 1.0
    iota = np.arange(V, dtype=np.float32).reshape(V, 1)
    in_maps = []
    for c in range(N_CORES):
        b, h = c // 2, c % 2
        dsl = slice(h * DH, (h + 1) * DH)
        in_maps.append({
            "ids_f": byte_ids[b].astype(np.float32).reshape(1, S),
            "iota_v": iota,
            "ones_r": np.ones((1, 128), np.float32),
            "ones_s": np.ones((1, TB), np.float32),
            "ones_c": np.ones((128, 1), np.float32),
            "embT": emb_byte,
            "embO": np.ascontiguousarray(emb_byte[:, dsl]),
            "posT": posT_full,
            "posO": np.ascontiguousarray(posT_full[dsl]),
            "wd_in": np.ascontiguousarray(Wd[:, :, dsl]),
            "bd_in": np.ascontiguousarray(bd[:, None, dsl]),
            "wbc_in": wbc,
            "bbc_in": bbc,
            "wdp_in": np.ascontiguousarray(WDp[:, :, dsl]),
            "bdp_in": np.ascontiguousarray(bDp[:, None, dsl]),
            "gb_in": gb,
            "gbo_in": np.ascontiguousarray(gb[:, :, dsl]),
            "gam_in": gamma[:, :, None],
            "gamo_in": np.ascontiguousarray(gamma[:, dsl, None]),
            "whT": Wh,
            "sel_in": sel,
            "bh_in": bh.reshape(1, V),
        })

    res = bass_utils.run_bass_kernel_spmd(nc, in_maps,
                                          core_ids=list(range(N_CORES)))
    out = np.empty((B, S, V), np.float32)
    for b in range(B):
        out[b] = res.results[2 * b]["logits_full"]
    return out

